# revision 5
# baseline (speedup 1.0000x reference)
"""Trainium2 Bass kernel for a quantized ResNet bottleneck block (dense_cnn).

Data-parallel over 8 NeuronCores: each core gets 8 of the 64 batch images;
conv weights are replicated. The fake-quant per-tensor scales are global
maxima over the full batch, computed with tiny cross-core AllReduce(max)
collectives on [128]-vectors of per-partition maxima.

All convolutions run on the PE array in bf16 on *integer-valued* quantized
operands (|q| <= 127, exactly representable in bf16), accumulated in fp32
PSUM, so the conv arithmetic is exact. Scales/biases are applied in fp32
epilogues; round-to-nearest-even is implemented with the +/-1.5*2^23 magic
trick (bit-exact, verified on hardware). The downsample-shortcut conv runs
in the dead time while the cross-core scale reductions for the main path
are in flight, streaming its result through small SBUF staging tiles to a
DRAM scratch buffer.
"""

import numpy as np

import concourse.bacc as bacc
import concourse.tile as tile
import concourse.mybir as mybir
import concourse.bass_isa as bass_isa
from concourse.bass_utils import run_bass_kernel_spmd

F32 = mybir.dt.float32
BF16 = mybir.dt.bfloat16
MAGIC = 12582912.0          # 1.5 * 2^23
INV127 = 1.0 / 127.0
N_CORES = 8
ALL_CORES = [list(range(N_CORES))]
AX = mybir.AxisListType
OP = mybir.AluOpType
AF = mybir.ActivationFunctionType

# model dims
CIN, P1, P3 = 512, 256, 1024
KC, MC1, MC3 = 4, 2, 8      # 128-channel chunks
H, W, OH, OW = 28, 28, 14, 14
HWF, OHWF = H * W, OH * OW


class _Cyc:
    """Weighted engine cycler: returns engine keys in proportion to weights."""

    def __init__(self, items):
        self.items = [(k, float(w)) for k, w in items]
        self.acc = [0.0] * len(items)

    def pick(self):
        i = min(range(len(self.items)),
                key=lambda j: (self.acc[j] + 1.0) / self.items[j][1])
        self.acc[i] += 1.0
        return self.items[i][0]


def _axis_for(ap):
    nfree = len(ap.shape) - 1
    return {1: AX.X, 2: AX.XY, 3: AX.XYZ, 4: AX.XYZW}[nfree]


def build(per=8):
    """Build the SPMD program for `per` images per core."""
    assert per % 2 == 0
    pairs = per // 2
    nc = bacc.Bacc("TRN2", target_bir_lowering=False, debug=False,
                   num_devices=N_CORES)

    x_d = nc.dram_tensor("x", [per, CIN, H, W], F32, kind="ExternalInput")
    w1t_d = nc.dram_tensor("w1t", [CIN, P1], F32, kind="ExternalInput")
    w2t_d = nc.dram_tensor("w2t", [P1, 9, P1], F32, kind="ExternalInput")
    w3t_d = nc.dram_tensor("w3t", [P1, P3], F32, kind="ExternalInput")
    wst_d = nc.dram_tensor("wst", [CIN, P3], F32, kind="ExternalInput")
    b1_d = nc.dram_tensor("b1", [P1], F32, kind="ExternalInput")
    b2_d = nc.dram_tensor("b2", [P1], F32, kind="ExternalInput")
    b3_d = nc.dram_tensor("b3", [P3], F32, kind="ExternalInput")
    bs_d = nc.dram_tensor("bs", [P3], F32, kind="ExternalInput")
    out_d = nc.dram_tensor("out", [per, P3, OH, OW], F32, kind="ExternalOutput")

    with tile.TileContext(nc, num_cores=N_CORES) as tc:
        _emit(nc, tc, per, pairs, x_d, w1t_d, w2t_d, w3t_d, wst_d,
              b1_d, b2_d, b3_d, bs_d, out_d)
    nc.compile()
    return nc


def _emit(nc, tc, per, pairs, x_d, w1t_d, w2t_d, w3t_d, wst_d,
          b1_d, b2_d, b3_d, bs_d, out_d):
    ctxs = []

    def open_pool(name, **kw):
        cm = tc.tile_pool(name=name, **kw)
        pool = cm.__enter__()
        ctxs.append((name, cm))
        return pool

    def close_pool(name):
        n, cm = ctxs.pop()
        assert n == name, f"LIFO violation: closing {name}, top is {n}"
        cm.__exit__(None, None, None)

    coeff = open_pool("coeff", bufs=1)
    dramp = open_pool("drb", bufs=1, space="DRAM")
    psp = open_pool("psum", bufs=8, space="PSUM")
    resst = open_pool("resst", bufs=1)
    xip = open_pool("xint", bufs=1)

    magic_t = coeff.tile([128, 1], F32, tag="magic")
    nc.gpsimd.memset(magic_t[:], MAGIC)
    res_dram = dramp.tile([128, MC3, per, OHWF], F32, tag="res_dram")

    # ---- elementwise helpers -------------------------------------------
    def emit_A(eng, dst, src, scale_ap):
        """dst = src*scale + MAGIC   (RNE rounds to integer at the add)"""
        if eng == "act":
            nc.scalar.activation(dst, src, AF.Identity,
                                 bias=magic_t[:], scale=scale_ap)
        elif eng == "dve":
            nc.vector.tensor_scalar(dst, src, scale_ap, MAGIC,
                                    op0=OP.mult, op1=OP.add)
        else:
            nc.gpsimd.tensor_scalar(dst, src, scale_ap, MAGIC,
                                    op0=OP.mult, op1=OP.add)

    def emit_B_sub(eng, dst, src):
        """dst = src - MAGIC  (bf16 integer out)"""
        e = nc.vector if eng == "dve" else nc.gpsimd
        e.tensor_scalar(dst, src, MAGIC, None, op0=OP.subtract)

    def emit_B_clip(eng, dst, src, m6_ap):
        """dst = min(src, M6) - MAGIC  (relu6 upper clip, bf16 out)"""
        e = nc.vector if eng == "dve" else nc.gpsimd
        e.tensor_scalar(dst, src, m6_ap, MAGIC, op0=OP.min, op1=OP.subtract)

    def scale_from_max(m_ap, tag):
        """s = max(m*(1/127), 1e-8); r = 1/s (IEEE). Returns (s, r)."""
        s = coeff.tile([128, 1], F32, tag=f"s_{tag}", name=f"s_{tag}")
        nc.vector.tensor_scalar(s[:], m_ap, INV127, 1e-8,
                                op0=OP.mult, op1=OP.max)
        r = coeff.tile([128, 1], F32, tag=f"r_{tag}", name=f"r_{tag}")
        nc.vector.reciprocal(r[:], s[:])
        return s, r

    def cross_core_max(lmax_ap, tag):
        """AllReduce-max [128,1] per-partition maxima across cores and
        partitions -> replicated [128,1]."""
        cin = dramp.tile([128], F32, tag=f"cin_{tag}", name=f"cin_{tag}")
        cout = dramp.tile([128], F32, tag=f"cout_{tag}", name=f"cout_{tag}")
        nc.sync.dma_start(cin[:], lmax_ap[:, 0])
        nc.gpsimd.collective_compute(
            "AllReduce", OP.max, replica_groups=ALL_CORES,
            ins=[cin.opt()], outs=[cout.opt()])
        g = coeff.tile([128, 1], F32, tag=f"g_{tag}", name=f"g_{tag}")
        nc.sync.dma_start(g[:, 0], cout[:])
        g2 = coeff.tile([128, 1], F32, tag=f"g2_{tag}", name=f"g2_{tag}")
        nc.gpsimd.partition_all_reduce(g2[:], g[:], 128, bass_isa.ReduceOp.max)
        return g2

    # ---- x load + local abs-max ----------------------------------------
    xfp = open_pool("xf32", bufs=1)
    xf = xfp.tile([128, KC, per, HWF], F32, tag="xf")
    x_re = x_d.ap().rearrange("b (kc c) h w -> c kc b (h w)", c=128)
    xmax = coeff.tile([128, KC * per], F32, tag="xmax")
    for kc in range(KC):
        for b in range(per):
            nc.sync.dma_start(xf[:, kc, b], x_re[:, kc, b])
    for kc in range(KC):
        for b in range(per):
            i = kc * per + b
            nc.vector.tensor_reduce(xmax[:, i:i + 1], xf[:, kc, b],
                                    axis=AX.X, op=OP.max,
                                    apply_absolute_value=True)

    # ---- weights: load + fake-quant (overlaps the x DMA) ---------------
    wsp = open_pool("wstage", bufs=1)

    def quant_weight(dram_ap, shape, tag):
        wf = wsp.tile(shape, F32, tag="wstage_big", name=f"wf_{tag}")
        nc.sync.dma_start(wf[:], dram_ap)
        lm = wsp.tile([128, 1], F32, tag=f"wm_{tag}", name=f"wm_{tag}")
        nc.vector.tensor_reduce(lm[:], wf[:], axis=_axis_for(wf[:]),
                                op=OP.max, apply_absolute_value=True)
        gm = wsp.tile([128, 1], F32, tag=f"wg_{tag}", name=f"wg_{tag}")
        nc.gpsimd.partition_all_reduce(gm[:], lm[:], 128, bass_isa.ReduceOp.max)
        s_w, r_w = scale_from_max(gm[:], f"w{tag}")
        emit_A("act", wf[:], wf[:], r_w[:])
        qw = coeff.tile(shape, BF16, tag=f"qw_{tag}", name=f"qw_{tag}")
        emit_B_sub("dve", qw[:], wf[:])
        return qw, s_w, r_w

    qw1, s_w1, r_w1 = quant_weight(
        w1t_d.ap().rearrange("(kc k) m -> k kc m", k=128), [128, KC, P1], "1")
    qw2, s_w2, r_w2 = quant_weight(
        w2t_d.ap().rearrange("(kc k) t m -> k kc t m", k=128),
        [128, MC1, 9, P1], "2")
    qw3, s_w3, r_w3 = quant_weight(
        w3t_d.ap().rearrange("(kc k) m -> k kc m", k=128), [128, MC1, P3], "3")
    qws, s_ws, r_ws = quant_weight(
        wst_d.ap().rearrange("(kc k) m -> k kc m", k=128), [128, KC, P3], "s")

    def quant_bias(dram_t, nmc, r_w, tag):
        """qb/s_w head-bias vector [128, nmc]."""
        bf = coeff.tile([128, nmc], F32, tag=f"bf_{tag}", name=f"bf_{tag}")
        nc.sync.dma_start(bf[:], dram_t.ap().rearrange("(mc c) -> c mc", c=128))
        lm = wsp.tile([128, 1], F32, tag=f"bm_{tag}", name=f"bm_{tag}")
        nc.vector.tensor_reduce(lm[:], bf[:], axis=AX.X, op=OP.max,
                                apply_absolute_value=True)
        gm = wsp.tile([128, 1], F32, tag=f"bg_{tag}", name=f"bg_{tag}")
        nc.gpsimd.partition_all_reduce(gm[:], lm[:], 128, bass_isa.ReduceOp.max)
        s_b, r_b = scale_from_max(gm[:], f"b{tag}")
        tq = wsp.tile([128, nmc], F32, tag=f"bt_{tag}", name=f"bt_{tag}")
        nc.vector.tensor_scalar(tq[:], bf[:], r_b[:], MAGIC,
                                op0=OP.mult, op1=OP.add)
        nc.vector.tensor_scalar(tq[:], tq[:], MAGIC, s_b[:],
                                op0=OP.subtract, op1=OP.mult)  # = qb
        qbs = coeff.tile([128, nmc], F32, tag=f"qbs_{tag}", name=f"qbs_{tag}")
        nc.vector.tensor_scalar(qbs[:], tq[:], r_w[:], None, op0=OP.mult)
        return qbs

    qbs1 = quant_bias(b1_d, MC1, r_w1, "1")
    qbs2 = quant_bias(b2_d, MC1, r_w2, "2")
    qbs3 = quant_bias(b3_d, MC3, r_w3, "3")
    qbsS0 = quant_bias(bs_d, MC3, r_ws, "s")
    qbsS = coeff.tile([128, MC3], F32, tag="qbsS")
    nc.vector.tensor_scalar(qbsS[:], qbsS0[:], s_ws[:], None, op0=OP.mult)

    close_pool("wstage")

    # ---- s_x and x quantization ----------------------------------------
    xm1 = coeff.tile([128, 1], F32, tag="xm1")
    nc.vector.tensor_reduce(xm1[:], xmax[:], axis=AX.X, op=OP.max)
    gx = cross_core_max(xm1[:], "x")
    s_x, r_x = scale_from_max(gx[:], "x")
    alpha_s = coeff.tile([128, 1], F32, tag="alpha_s")
    nc.vector.tensor_tensor(alpha_s[:], s_x[:], s_ws[:], op=OP.mult)

    xi = xip.tile([128, KC, per, H, W], BF16, tag="xi")
    cycA = _Cyc([("act", 5.0), ("gp", 3.0)])
    cycB = _Cyc([("dve", 5.0), ("gp", 2.0)])
    for kc in range(KC):
        for b in range(per):
            emit_A(cycA.pick(), xf[:, kc, b], xf[:, kc, b], r_x[:])
            emit_B_sub(cycB.pick(), xi[:, kc, b], xf[:, kc, b])
    close_pool("xf32")

    # ---- persistent mid buffers ----------------------------------------
    d2p = open_pool("d2i2", bufs=1)
    d2 = d2p.tile([128, MC1, per, OHWF], F32, tag="d2")
    i2 = d2p.tile([128, MC1, per, OHWF], BF16, tag="i2")
    i1pool = open_pool("i1pad", bufs=1)
    i1 = i1pool.tile([128, MC1, per, H + 2, W + 2], BF16, tag="i1")
    nc.gpsimd.memset(i1[:], 0.0)

    # ---- conv1 (1x1, 512->256) + heads + maxes -------------------------
    d1p = open_pool("d1", bufs=1)
    d1 = d1p.tile([128, MC1, per, HWF], F32, tag="d1")
    o1max = coeff.tile([128, MC1 * per], F32, tag="o1max")

    quad = 4 if per % 4 == 0 else 2
    for mc in range(MC1):
        for q0 in range(0, per, quad):
            pst = {}
            for kc in range(KC):
                for ii in range(quad):
                    b = q0 + ii
                    for hf in range(2):
                        if kc == 0:
                            pst[(ii, hf)] = psp.tile(
                                [128, 392], F32, tag="ps",
                                name=f"ps1_{mc}_{q0}_{ii}_{hf}")
                        nc.tensor.matmul(
                            pst[(ii, hf)][:],
                            qw1[:, kc, mc * 128:(mc + 1) * 128],
                            xi[:, kc, b, hf * 14:(hf + 1) * 14, :],
                            start=(kc == 0), stop=(kc == KC - 1))
            for ii in range(quad):
                b = q0 + ii
                for hf in range(2):
                    nc.scalar.activation(
                        d1[:, mc, b, hf * 392:(hf + 1) * 392],
                        pst[(ii, hf)][:], AF.Relu,
                        bias=qbs1[:, mc:mc + 1], scale=s_x[:])
            for ii in range(quad):
                b = q0 + ii
                i = mc * per + b
                nc.vector.tensor_reduce(o1max[:, i:i + 1], d1[:, mc, b],
                                        axis=AX.X, op=OP.max)

    # ---- shortcut conv part A (PE filler during s_1 reduction) ---------
    def shortcut_chunk(mcs):
        for mc in mcs:
            pst = {}
            for kc in range(KC):
                for p in range(pairs):
                    if kc == 0:
                        pst[p] = psp.tile([128, 2, OHWF], F32, tag="ps",
                                          name=f"pss_{mc}_{p}")
                    nc.tensor.matmul(
                        pst[p][:],
                        qws[:, kc, mc * 128:(mc + 1) * 128],
                        xi[:, kc, 2 * p:2 * p + 2, 0:27:2, 0:27:2],
                        start=(kc == 0), stop=(kc == KC - 1))
            for p in range(pairs):
                st = resst.tile([128, 2, OHWF], F32, tag="resst", bufs=4,
                                name=f"resst_{mc}_{p}")
                nc.scalar.activation(
                    st[:], pst[p][:], AF.Identity,
                    bias=qbsS[:, mc:mc + 1], scale=alpha_s[:])
                nc.sync.dma_start(res_dram[:, mc, 2 * p:2 * p + 2, :], st[:])

    shortcut_chunk(range(0, MC3 // 2))

    # ---- s_1 chain ------------------------------------------------------
    o1m = coeff.tile([128, 1], F32, tag="o1m")
    nc.vector.tensor_reduce(o1m[:], o1max[:], axis=AX.X, op=OP.max)
    g1 = cross_core_max(o1m[:], "1")
    e1max = coeff.tile([128, 1], F32, tag="e1max")
    nc.vector.tensor_tensor(e1max[:], g1[:], s_w1[:], op=OP.mult)
    f1max = coeff.tile([128, 1], F32, tag="f1max")
    nc.vector.tensor_scalar(f1max[:], e1max[:], 6.0, None, op0=OP.min)
    s_1, r_1 = scale_from_max(f1max[:], "1")
    k1 = coeff.tile([128, 1], F32, tag="k1")
    nc.vector.tensor_tensor(k1[:], s_w1[:], r_1[:], op=OP.mult)
    m6_1 = coeff.tile([128, 1], F32, tag="m6_1")
    nc.vector.tensor_scalar(m6_1[:], r_1[:], 6.0, MAGIC,
                            op0=OP.mult, op1=OP.add)

    # ---- o1 quant -> i1 (padded interior) -------------------------------
    cycA1 = _Cyc([("act", 5.0), ("gp", 3.0), ("dve", 3.0)])
    cycB1 = _Cyc([("dve", 5.0), ("gp", 2.0)])
    for b in range(per):
        for mc in range(MC1):
            emit_A(cycA1.pick(), d1[:, mc, b], d1[:, mc, b], k1[:])
            emit_B_clip(cycB1.pick(), i1[:, mc, b, 1:29, 1:29],
                        d1[:, mc, b], m6_1[:])
    close_pool("d1")

    # ---- conv2 (3x3 stride 2, 256->256) --------------------------------
    o2max = coeff.tile([128, MC1 * pairs], F32, tag="o2max")
    taps = [(dy, dx) for dy in range(3) for dx in range(3)]
    for mc in range(MC1):
        pst = {}
        for kc in range(MC1):
            for ti, (dy, dx) in enumerate(taps):
                for p in range(pairs):
                    if kc == 0 and ti == 0:
                        pst[p] = psp.tile([128, 2, OH, OW], F32, tag="ps",
                                          name=f"ps2_{mc}_{p}")
                    nc.tensor.matmul(
                        pst[p][:],
                        qw2[:, kc, ti, mc * 128:(mc + 1) * 128],
                        i1[:, kc, 2 * p:2 * p + 2, dy:dy + 28:2, dx:dx + 28:2],
                        start=(kc == 0 and ti == 0),
                        stop=(kc == MC1 - 1 and ti == 8))
        for p in range(pairs):
            nc.scalar.activation(
                d2[:, mc, 2 * p:2 * p + 2, :], pst[p][:], AF.Relu,
                bias=qbs2[:, mc:mc + 1], scale=s_1[:])
        for p in range(pairs):
            i = mc * pairs + p
            nc.vector.tensor_reduce(o2max[:, i:i + 1],
                                    d2[:, mc, 2 * p:2 * p + 2, :],
                                    axis=AX.XY, op=OP.max)

    # ---- shortcut conv part B (PE filler during s_2 reduction) ---------
    shortcut_chunk(range(MC3 // 2, MC3))

    # ---- s_2 chain + o2 quant ------------------------------------------
    o2m = coeff.tile([128, 1], F32, tag="o2m")
    nc.vector.tensor_reduce(o2m[:], o2max[:], axis=AX.X, op=OP.max)
    g2m = cross_core_max(o2m[:], "2")
    e2max = coeff.tile([128, 1], F32, tag="e2max")
    nc.vector.tensor_tensor(e2max[:], g2m[:], s_w2[:], op=OP.mult)
    f2max = coeff.tile([128, 1], F32, tag="f2max")
    nc.vector.tensor_scalar(f2max[:], e2max[:], 6.0, None, op0=OP.min)
    s_2, r_2 = scale_from_max(f2max[:], "2")
    k2 = coeff.tile([128, 1], F32, tag="k2")
    nc.vector.tensor_tensor(k2[:], s_w2[:], r_2[:], op=OP.mult)
    m6_2 = coeff.tile([128, 1], F32, tag="m6_2")
    nc.vector.tensor_scalar(m6_2[:], r_2[:], 6.0, MAGIC,
                            op0=OP.mult, op1=OP.add)

    cycA2 = _Cyc([("act", 5.0), ("gp", 3.0), ("dve", 3.0)])
    cycB2 = _Cyc([("dve", 5.0), ("gp", 2.0)])
    for p in range(pairs):
        for mc in range(MC1):
            sl = (slice(None), mc, slice(2 * p, 2 * p + 2), slice(None))
            emit_A(cycA2.pick(), d2[sl], d2[sl], k2[:])
            emit_B_clip(cycB2.pick(), i2[sl], d2[sl], m6_2[:])

    # ---- conv3 (1x1, 256->1024) ----------------------------------------
    d3p = open_pool("d3", bufs=1)
    d3 = d3p.tile([128, MC3, per, OHWF], F32, tag="d3")
    o3max = coeff.tile([128, MC3 * pairs], F32, tag="o3max")
    for mc in range(MC3):
        pst = {}
        for kc in range(MC1):
            for p in range(pairs):
                if kc == 0:
                    pst[p] = psp.tile([128, 2, OHWF], F32, tag="ps",
                                      name=f"ps3_{mc}_{p}")
                nc.tensor.matmul(
                    pst[p][:],
                    qw3[:, kc, mc * 128:(mc + 1) * 128],
                    i2[:, kc, 2 * p:2 * p + 2, :],
                    start=(kc == 0), stop=(kc == MC1 - 1))
        for p in range(pairs):
            nc.scalar.activation(
                d3[:, mc, 2 * p:2 * p + 2, :], pst[p][:], AF.Identity,
                bias=qbs3[:, mc:mc + 1], scale=s_2[:])
        for p in range(pairs):
            i = mc * pairs + p
            nc.vector.tensor_reduce(o3max[:, i:i + 1],
                                    d3[:, mc, 2 * p:2 * p + 2, :],
                                    axis=AX.XY, op=OP.max,
                                    apply_absolute_value=True)

    # ---- s_3 chain ------------------------------------------------------
    o3m = coeff.tile([128, 1], F32, tag="o3m")
    nc.vector.tensor_reduce(o3m[:], o3max[:], axis=AX.X, op=OP.max)
    g3 = cross_core_max(o3m[:], "3")
    v3max = coeff.tile([128, 1], F32, tag="v3max")
    nc.vector.tensor_tensor(v3max[:], g3[:], s_w3[:], op=OP.mult)
    s_3, r_3 = scale_from_max(v3max[:], "3")
    k3 = coeff.tile([128, 1], F32, tag="k3")
    nc.vector.tensor_tensor(k3[:], s_w3[:], r_3[:], op=OP.mult)

    # ---- final: quantize conv3, add residual, relu6, store -------------
    out_re = out_d.ap().rearrange("b (mc c) h w -> c mc b (h w)", c=128)
    cycA3 = _Cyc([("act", 5.0), ("gp", 3.0)])
    cycC3 = _Cyc([("dve", 5.0), ("gp", 3.0)])
    cycD3 = _Cyc([("dve", 1.0), ("gp", 1.0)])
    for p in range(pairs):
        for mc in range(MC3):
            sl = (slice(None), mc, slice(2 * p, 2 * p + 2), slice(None))
            rt = resst.tile([128, 2, OHWF], F32, tag="resin", bufs=4,
                            name=f"resin_{mc}_{p}")
            nc.sync.dma_start(rt[:], res_dram[:, mc, 2 * p:2 * p + 2, :])
            emit_A(cycA3.pick(), d3[sl], d3[sl], k3[:])
            nc.vector.tensor_scalar(d3[sl], d3[sl], MAGIC, s_3[:],
                                    op0=OP.subtract, op1=OP.mult)
            e = nc.vector if cycC3.pick() == "dve" else nc.gpsimd
            e.tensor_tensor(d3[sl], d3[sl], rt[:], op=OP.add)
            e = nc.vector if cycD3.pick() == "dve" else nc.gpsimd
            e.tensor_scalar(d3[sl], d3[sl], 6.0, 0.0, op0=OP.min, op1=OP.max)
            nc.sync.dma_start(out_re[sl], d3[sl])

    close_pool("d3")
    close_pool("i1pad")
    close_pool("d2i2")
    close_pool("xint")
    close_pool("resst")
    close_pool("psum")
    close_pool("drb")
    close_pool("coeff")


_NC_CACHE = {}


def _get_nc(per):
    if per not in _NC_CACHE:
        _NC_CACHE[per] = build(per)
    return _NC_CACHE[per]


def make_in_maps(x, w1, b1, w2, b2, w3, b3, ws, bs, per):
    """Host-side layout prep (transposes/sharding only, no math)."""
    n_sh = x.shape[0] // per
    w1t = np.ascontiguousarray(w1.reshape(P1, CIN).T)
    w2t = np.ascontiguousarray(w2.reshape(P1, P1, 9).transpose(1, 2, 0))
    w3t = np.ascontiguousarray(w3.reshape(P3, P1).T)
    wst = np.ascontiguousarray(ws.reshape(P3, CIN).T)
    common = {"w1t": w1t, "w2t": w2t, "w3t": w3t, "wst": wst,
              "b1": b1, "b2": b2, "b3": b3, "bs": bs}
    return [{"x": np.ascontiguousarray(x[i * per:(i + 1) * per]), **common}
            for i in range(n_sh)]


def kernel(x, w1, b1, w2, b2, w3, b3, ws, bs):
    x = np.asarray(x, dtype=np.float32)
    per = x.shape[0] // N_CORES
    nc = _get_nc(per)
    in_maps = make_in_maps(np.asarray(x), np.asarray(w1), np.asarray(b1),
                           np.asarray(w2), np.asarray(b2), np.asarray(w3),
                           np.asarray(b3), np.asarray(ws), np.asarray(bs), per)
    res = run_bass_kernel_spmd(nc, in_maps, list(range(N_CORES)))
    return np.concatenate([res.results[i]["out"] for i in range(N_CORES)],
                          axis=0).astype(np.float32)
